# revision 9
# baseline (speedup 1.0000x reference)
"""HADAR decomposer kernel for 8 Trainium2 NeuronCores.

Per-pixel 120-way nearest-candidate search (6 materials x 20 temperatures)
+ gather of per-pixel outputs, data-parallel over the pixel dim.

Device layout is channel-major ("transposed"): inputs/outputs are [128, Np/2]
per core with two 55-row pixel-chunks at partition bases 0 and 64 so every
DMA uses ~128 partitions and the PE can row/col-pack pairs of matmuls.
"""

import os
import sys

for _p in ("/opt/trn_rl_repo", "/root/.axon_site/_ro/trn_rl_repo"):
    if os.path.isdir(_p) and _p not in sys.path:
        sys.path.insert(0, _p)

import numpy as np

import concourse.bass as bass
import concourse.bacc as bacc
import concourse.mybir as mybir
from concourse import bass_isa
from concourse.bass_utils import run_bass_kernel_spmd
from concourse.tile import TileContext

# Problem constants (hardcoded per the harness contract).
N_CORES = 8
N = 1048576
C = 54
M_LIB = 6
N_TEMPS = 20
K = M_LIB * N_TEMPS  # 120 candidates
NP_CORE = N // N_CORES  # 131072 pixels per core
NH = NP_CORE // 2  # 65536 pixels per partition-chunk
F = 512  # pixels per PE tile (one PSUM bank at fp32)
NT = NH // F  # 128 tiles per core

C1 = 1.191042e-8
C2 = 1.4387752

F32 = mybir.dt.float32
F32R = mybir.dt.float32r  # native-rate fp32 matmul dtype on trn2

LAST_EXEC_NS = None  # set when BASS_TRACE=1

_program = None


def _install_trace_hook():
    """Make NTFF profiling work when the image's antenv lacks axon_hooks."""
    try:
        from antenv.axon_hooks import get_axon_ntff_profile_hook  # noqa: F401

        return
    except ImportError:
        pass
    try:
        import types

        import antenv
        from trn_agent_boot.trn_boot import _ntff_profile_via_ctypes

        hook = _ntff_profile_via_ctypes("/opt/axon/libaxon_pjrt.so")
        mod = types.ModuleType("antenv.axon_hooks")
        _state = {"hook": hook}
        mod.set_axon_ntff_profile_hook = lambda h: _state.__setitem__("hook", h)
        mod.get_axon_ntff_profile_hook = lambda: _state["hook"]
        sys.modules["antenv.axon_hooks"] = mod
        antenv.axon_hooks = mod
    except Exception:
        pass


def _build_program():
    global _program
    if _program is not None:
        return _program

    nc = bacc.Bacc()
    xT = nc.dram_tensor("xT", [128, NH], F32, kind="ExternalInput")
    vcand = nc.dram_tensor("vcand", [128, K], F32, kind="ExternalInput")
    tableAB = nc.dram_tensor("tableAB", [K, 128], F32R, kind="ExternalInput")
    outA = nc.dram_tensor("outA", [128, NH], F32, kind="ExternalOutput")
    outB = nc.dram_tensor("outB", [128, NH], F32, kind="ExternalOutput")

    with TileContext(nc) as tc:
        with (
            tc.tile_pool(name="const", bufs=1) as cpool,
            tc.tile_pool(name="xin", bufs=4) as xpool,
            tc.tile_pool(name="work", bufs=2) as wpool,
            tc.tile_pool(name="outs", bufs=3) as opool,
            tc.tile_pool(name="psum", bufs=2, space="PSUM") as ppool,
        ):
            vc = cpool.tile([128, K], F32)
            nc.sync.dma_start(out=vc[:, :], in_=vcand[:, :])
            tAB = cpool.tile([K, 128], F32R)
            nc.sync.dma_start(out=tAB[:, :], in_=tableAB[:, :])

            for i in range(NT):
                sl = bass.ts(i, F)
                xt = xpool.tile([128, F], F32, tag="xt")
                nc.sync.dma_start(out=xt[:, :], in_=xT[:, sl])

                # q[k, n] = 2*dot(s_n, cand_k) - ||cand_k||^2  (ones-row bias)
                qA = ppool.tile([K, F], F32, tag="qA")
                qB = ppool.tile([K, F], F32, tag="qB")
                nc.tensor.matmul(
                    qA[:, :], lhsT=vc[0:55, :], rhs=xt[0:55, :], start=True, stop=True
                )
                nc.tensor.matmul(
                    qB[:, :], lhsT=vc[64:119, :], rhs=xt[64:119, :], start=True, stop=True
                )

                qsA = wpool.tile([K, F], F32, tag="qsA")
                nc.scalar.copy(out=qsA[:, :], in_=qA[:, :])
                qsB = wpool.tile([K, F], F32, tag="qsB")
                nc.scalar.copy(out=qsB[:, :], in_=qB[:, :])

                mA = wpool.tile([K, F], F32, tag="mA")
                nc.gpsimd.partition_all_reduce(
                    mA[:, :], qsA[:, :], K, bass_isa.ReduceOp.max
                )
                mB = wpool.tile([K, F], F32, tag="mB")
                nc.gpsimd.partition_all_reduce(
                    mB[:, :], qsB[:, :], K, bass_isa.ReduceOp.max
                )

                ohA = wpool.tile([K, F], F32R, tag="ohA")
                nc.vector.tensor_tensor(
                    ohA[:, :], qsA[:, :], mA[:, :], mybir.AluOpType.is_equal
                )
                ohB = wpool.tile([K, F], F32R, tag="ohB")
                nc.vector.tensor_tensor(
                    ohB[:, :], qsB[:, :], mB[:, :], mybir.AluOpType.is_equal
                )

                # gather (per chunk, M=128): rows 0-53 recon, 54 best_t,
                # 55 texture, 64-117 best_e, 118 count
                oAp = ppool.tile([128, F], F32, tag="oAp")
                nc.tensor.matmul(
                    oAp[:, :], lhsT=tAB[:, :], rhs=ohA[:, :], start=True, stop=True
                )
                oBp = ppool.tile([128, F], F32, tag="oBp")
                nc.tensor.matmul(
                    oBp[:, :], lhsT=tAB[:, :], rhs=ohB[:, :], start=True, stop=True
                )

                oAs = opool.tile([128, F], F32, tag="oAs")
                nc.vector.tensor_copy(oAs[:, :], oAp[:, :])
                oBs = opool.tile([128, F], F32, tag="oBs")
                nc.scalar.copy(out=oBs[:, :], in_=oBp[:, :])

                nc.sync.dma_start(out=outA[:, sl], in_=oAs[:, :])
                nc.sync.dma_start(out=outB[:, sl], in_=oBs[:, :])

    nc.finalize()  # runs Bacc passes (sync-wait splitting, event sems, regs)
    _program = nc
    return nc


def _host_tables(s_sky, s_ground, library, wg):
    """Candidate model spectra and gather tables, f32 to mirror reference."""
    t_cand = np.linspace(250.0, 350.0, N_TEMPS, dtype=np.float32)
    x_amb = (
        np.float32(0.5) * s_sky.astype(np.float32)
        + np.float32(0.5) * s_ground.astype(np.float32)
    )
    nu = wg.astype(np.float32)[None, :]
    B = (
        np.float32(C1)
        * nu**3
        / np.expm1(np.float32(C2) * nu / t_cand[:, None].astype(np.float32))
    ).astype(np.float32)  # [T, C]
    lib = library.astype(np.float32)
    cand = (lib[:, None, :] * B[None, :, :] + (1.0 - lib[:, None, :]) * x_amb).astype(
        np.float32
    )
    cand = cand.reshape(K, C)  # k = m*N_TEMPS + t (matches reference)
    c2 = np.sum(cand.astype(np.float32) ** 2, axis=1).astype(np.float32)

    # lhsT for q-matmul: rows 0..53 = 2*cand^T, row 54 = -c2
    V = np.zeros((55, K), dtype=np.float32)
    V[0:54, :] = (2.0 * cand.T).astype(np.float32)
    V[54, :] = -c2
    vcand = np.zeros((128, K), dtype=np.float32)
    vcand[0:55, :] = V
    vcand[64:119, :] = V

    m_idx = np.arange(K) // N_TEMPS
    t_idx = np.arange(K) % N_TEMPS
    tableAB = np.zeros((K, 128), dtype=np.float32)
    tableAB[:, 0:54] = cand
    tableAB[:, 54] = t_cand[t_idx]
    tableAB[:, 55] = lib[m_idx].mean(axis=1)
    tableAB[:, 64:118] = lib[m_idx]
    tableAB[:, 118] = 1.0

    return t_cand, x_amb, cand, c2, vcand, tableAB


def kernel(s_obs, s_sky, s_ground, library, wavenumber_grid):
    global LAST_EXEC_NS
    s_obs = np.ascontiguousarray(np.asarray(s_obs, dtype=np.float32))
    s_sky = np.asarray(s_sky, dtype=np.float32)
    s_ground = np.asarray(s_ground, dtype=np.float32)
    library = np.asarray(library, dtype=np.float32)
    wg = np.asarray(wavenumber_grid, dtype=np.float32)
    assert s_obs.shape == (N, C)

    t_cand, x_amb, cand, c2, vcand, tableAB = _host_tables(
        s_sky, s_ground, library, wg
    )

    # Shard + transpose input: per core [128, NH] with chunks at rows 0/64
    # (54 channel rows + a ones row each).
    sT = s_obs.T  # [C, N] view
    in_maps = []
    for c in range(N_CORES):
        x = np.zeros((128, NH), dtype=np.float32)
        base = c * NP_CORE
        x[0:54, :] = sT[:, base : base + NH]
        x[54, :] = 1.0
        x[64:118, :] = sT[:, base + NH : base + NP_CORE]
        x[118, :] = 1.0
        in_maps.append(
            {"xT": x, "vcand": vcand, "tableAB": tableAB}
        )

    trace = bool(os.environ.get("BASS_TRACE"))
    if trace:
        _install_trace_hook()
        import concourse.bass_utils as _bu

        _bu.upload_artifacts = lambda d: d  # no egress in this sandbox
    nc = _build_program()
    res = run_bass_kernel_spmd(
        nc,
        in_maps,
        core_ids=list(range(N_CORES)),
        trace=trace,
    )
    LAST_EXEC_NS = res.exec_time_ns

    best_t = np.empty(N, dtype=np.float32)
    texture = np.empty(N, dtype=np.float32)
    count = np.empty(N, dtype=np.float32)
    s_recon = np.empty((N, C), dtype=np.float32)
    best_e = np.empty((N, C), dtype=np.float32)
    for c in range(N_CORES):
        base = c * NP_CORE
        for half, r in ((0, res.results[c]["outA"]), (1, res.results[c]["outB"])):
            lo = base + half * NH
            hi = lo + NH
            s_recon[lo:hi] = r[0:54].T
            best_t[lo:hi] = r[54]
            texture[lo:hi] = r[55]
            best_e[lo:hi] = r[64:118].T
            count[lo:hi] = r[118]

    # Exact-tie repair: pixels where the one-hot had != 1 entries get the
    # reference argmin recomputed on host (count is exact: ties are rare).
    bad = np.flatnonzero(count != 1.0)
    if bad.size:
        d = s_obs[bad]
        s2 = np.sum(d.astype(np.float32) ** 2, axis=1, keepdims=True)
        losses = (s2 - 2.0 * (d @ cand.T) + c2[None, :]).astype(np.float32)
        kfix = np.argmin(losses, axis=1)
        s_recon[bad] = cand[kfix]
        best_t[bad] = t_cand[kfix % N_TEMPS]
        best_e[bad] = library[kfix // N_TEMPS]
        texture[bad] = library[kfix // N_TEMPS].mean(axis=1)

    best_v = np.full(N, 0.5, dtype=np.float32)
    beta = np.zeros((N, 1), dtype=np.float32)
    diff = s_obs.astype(np.float64) - s_recon.astype(np.float64)
    objective = np.float32(np.mean(np.sum(diff * diff, axis=1)))

    return best_t, best_e, best_v, beta, texture, s_recon, objective


# revision 10
# speedup vs baseline: 1.0073x; 1.0073x over previous
"""HADAR decomposer kernel for 8 Trainium2 NeuronCores.

Per-pixel 120-way nearest-candidate search (6 materials x 20 temperatures)
+ gather of per-pixel outputs, data-parallel over the pixel dim.

Device layout is channel-major ("transposed"): inputs/outputs are [128, Np/2]
per core with two 55-row pixel-chunks at partition bases 0 and 64 so every
DMA uses ~128 partitions and the PE can row/col-pack pairs of matmuls.
"""

import os
import sys

for _p in ("/opt/trn_rl_repo", "/root/.axon_site/_ro/trn_rl_repo"):
    if os.path.isdir(_p) and _p not in sys.path:
        sys.path.insert(0, _p)

import numpy as np

import concourse.bass as bass
import concourse.bacc as bacc
import concourse.mybir as mybir
from concourse import bass_isa
from concourse.bass_utils import run_bass_kernel_spmd
from concourse.tile import TileContext

# Problem constants (hardcoded per the harness contract).
N_CORES = 8
N = 1048576
C = 54
M_LIB = 6
N_TEMPS = 20
K = M_LIB * N_TEMPS  # 120 candidates
NP_CORE = N // N_CORES  # 131072 pixels per core
NH = NP_CORE // 2  # 65536 pixels per partition-chunk
F = 512  # pixels per PE tile (one PSUM bank at fp32)
NT = NH // F  # 128 tiles per core

C1 = 1.191042e-8
C2 = 1.4387752

F32 = mybir.dt.float32
F32R = mybir.dt.float32r  # native-rate fp32 matmul dtype on trn2

LAST_EXEC_NS = None  # set when BASS_TRACE=1

_program = None


def _install_trace_hook():
    """Make NTFF profiling work when the image's antenv lacks axon_hooks."""
    try:
        from antenv.axon_hooks import get_axon_ntff_profile_hook  # noqa: F401

        return
    except ImportError:
        pass
    try:
        import types

        import antenv
        from trn_agent_boot.trn_boot import _ntff_profile_via_ctypes

        hook = _ntff_profile_via_ctypes("/opt/axon/libaxon_pjrt.so")
        mod = types.ModuleType("antenv.axon_hooks")
        _state = {"hook": hook}
        mod.set_axon_ntff_profile_hook = lambda h: _state.__setitem__("hook", h)
        mod.get_axon_ntff_profile_hook = lambda: _state["hook"]
        sys.modules["antenv.axon_hooks"] = mod
        antenv.axon_hooks = mod
    except Exception:
        pass


def _build_program():
    global _program
    if _program is not None:
        return _program

    nc = bacc.Bacc()
    xT = nc.dram_tensor("xT", [128, NH], F32, kind="ExternalInput")
    vcand = nc.dram_tensor("vcand", [128, K], F32, kind="ExternalInput")
    tableAB = nc.dram_tensor("tableAB", [K, 128], F32R, kind="ExternalInput")
    outA = nc.dram_tensor("outA", [128, NH], F32, kind="ExternalOutput")
    outB = nc.dram_tensor("outB", [128, NH], F32, kind="ExternalOutput")

    with TileContext(nc) as tc:
        with (
            tc.tile_pool(name="const", bufs=1) as cpool,
            tc.tile_pool(name="xin", bufs=8) as xpool,
            tc.tile_pool(name="work", bufs=4) as wpool,
            tc.tile_pool(name="outs", bufs=6) as opool,
            tc.tile_pool(name="psum", bufs=2, space="PSUM") as ppool,
        ):
            vc = cpool.tile([128, K], F32)
            nc.sync.dma_start(out=vc[:, :], in_=vcand[:, :])
            tAB = cpool.tile([K, 128], F32R)
            nc.sync.dma_start(out=tAB[:, :], in_=tableAB[:, :])

            for i in range(NT):
                sl = bass.ts(i, F)
                xt = xpool.tile([128, F], F32, tag="xt")
                nc.sync.dma_start(out=xt[:, :], in_=xT[:, sl])

                # q[k, n] = 2*dot(s_n, cand_k) - ||cand_k||^2  (ones-row bias)
                qA = ppool.tile([K, F], F32, tag="qA")
                qB = ppool.tile([K, F], F32, tag="qB")
                nc.tensor.matmul(
                    qA[:, :], lhsT=vc[0:55, :], rhs=xt[0:55, :], start=True, stop=True
                )
                nc.tensor.matmul(
                    qB[:, :], lhsT=vc[64:119, :], rhs=xt[64:119, :], start=True, stop=True
                )

                qsA = wpool.tile([K, F], F32, tag="qsA")
                nc.scalar.copy(out=qsA[:, :], in_=qA[:, :])
                qsB = wpool.tile([K, F], F32, tag="qsB")
                nc.scalar.copy(out=qsB[:, :], in_=qB[:, :])

                mA = wpool.tile([K, F], F32, tag="mA")
                nc.gpsimd.partition_all_reduce(
                    mA[:, :], qsA[:, :], K, bass_isa.ReduceOp.max
                )
                mB = wpool.tile([K, F], F32, tag="mB")
                nc.gpsimd.partition_all_reduce(
                    mB[:, :], qsB[:, :], K, bass_isa.ReduceOp.max
                )

                ohA = wpool.tile([K, F], F32R, tag="ohA")
                nc.vector.tensor_tensor(
                    ohA[:, :], qsA[:, :], mA[:, :], mybir.AluOpType.is_equal
                )
                ohB = wpool.tile([K, F], F32R, tag="ohB")
                nc.vector.tensor_tensor(
                    ohB[:, :], qsB[:, :], mB[:, :], mybir.AluOpType.is_equal
                )

                # gather (per chunk, M=128): rows 0-53 recon, 54 best_t,
                # 55 texture, 64-117 best_e, 118 count
                oAp = ppool.tile([128, F], F32, tag="oAp")
                nc.tensor.matmul(
                    oAp[:, :], lhsT=tAB[:, :], rhs=ohA[:, :], start=True, stop=True
                )
                oBp = ppool.tile([128, F], F32, tag="oBp")
                nc.tensor.matmul(
                    oBp[:, :], lhsT=tAB[:, :], rhs=ohB[:, :], start=True, stop=True
                )

                oAs = opool.tile([128, F], F32, tag="oAs")
                nc.vector.tensor_copy(oAs[:, :], oAp[:, :])
                oBs = opool.tile([128, F], F32, tag="oBs")
                nc.scalar.copy(out=oBs[:, :], in_=oBp[:, :])

                nc.sync.dma_start(out=outA[:, sl], in_=oAs[:, :])
                nc.sync.dma_start(out=outB[:, sl], in_=oBs[:, :])

    nc.finalize()  # runs Bacc passes (sync-wait splitting, event sems, regs)
    _program = nc
    return nc


def _host_tables(s_sky, s_ground, library, wg):
    """Candidate model spectra and gather tables, f32 to mirror reference."""
    t_cand = np.linspace(250.0, 350.0, N_TEMPS, dtype=np.float32)
    x_amb = (
        np.float32(0.5) * s_sky.astype(np.float32)
        + np.float32(0.5) * s_ground.astype(np.float32)
    )
    nu = wg.astype(np.float32)[None, :]
    B = (
        np.float32(C1)
        * nu**3
        / np.expm1(np.float32(C2) * nu / t_cand[:, None].astype(np.float32))
    ).astype(np.float32)  # [T, C]
    lib = library.astype(np.float32)
    cand = (lib[:, None, :] * B[None, :, :] + (1.0 - lib[:, None, :]) * x_amb).astype(
        np.float32
    )
    cand = cand.reshape(K, C)  # k = m*N_TEMPS + t (matches reference)
    c2 = np.sum(cand.astype(np.float32) ** 2, axis=1).astype(np.float32)

    # lhsT for q-matmul: rows 0..53 = 2*cand^T, row 54 = -c2
    V = np.zeros((55, K), dtype=np.float32)
    V[0:54, :] = (2.0 * cand.T).astype(np.float32)
    V[54, :] = -c2
    vcand = np.zeros((128, K), dtype=np.float32)
    vcand[0:55, :] = V
    vcand[64:119, :] = V

    m_idx = np.arange(K) // N_TEMPS
    t_idx = np.arange(K) % N_TEMPS
    tableAB = np.zeros((K, 128), dtype=np.float32)
    tableAB[:, 0:54] = cand
    tableAB[:, 54] = t_cand[t_idx]
    tableAB[:, 55] = lib[m_idx].mean(axis=1)
    tableAB[:, 64:118] = lib[m_idx]
    tableAB[:, 118] = 1.0

    return t_cand, x_amb, cand, c2, vcand, tableAB


def kernel(s_obs, s_sky, s_ground, library, wavenumber_grid):
    global LAST_EXEC_NS
    s_obs = np.ascontiguousarray(np.asarray(s_obs, dtype=np.float32))
    s_sky = np.asarray(s_sky, dtype=np.float32)
    s_ground = np.asarray(s_ground, dtype=np.float32)
    library = np.asarray(library, dtype=np.float32)
    wg = np.asarray(wavenumber_grid, dtype=np.float32)
    assert s_obs.shape == (N, C)

    t_cand, x_amb, cand, c2, vcand, tableAB = _host_tables(
        s_sky, s_ground, library, wg
    )

    # Shard + transpose input: per core [128, NH] with chunks at rows 0/64
    # (54 channel rows + a ones row each).
    sT = s_obs.T  # [C, N] view
    in_maps = []
    for c in range(N_CORES):
        x = np.zeros((128, NH), dtype=np.float32)
        base = c * NP_CORE
        x[0:54, :] = sT[:, base : base + NH]
        x[54, :] = 1.0
        x[64:118, :] = sT[:, base + NH : base + NP_CORE]
        x[118, :] = 1.0
        in_maps.append(
            {"xT": x, "vcand": vcand, "tableAB": tableAB}
        )

    trace = bool(os.environ.get("BASS_TRACE"))
    if trace:
        _install_trace_hook()
        import concourse.bass_utils as _bu

        _bu.upload_artifacts = lambda d: d  # no egress in this sandbox
    nc = _build_program()
    res = run_bass_kernel_spmd(
        nc,
        in_maps,
        core_ids=list(range(N_CORES)),
        trace=trace,
    )
    LAST_EXEC_NS = res.exec_time_ns

    best_t = np.empty(N, dtype=np.float32)
    texture = np.empty(N, dtype=np.float32)
    count = np.empty(N, dtype=np.float32)
    s_recon = np.empty((N, C), dtype=np.float32)
    best_e = np.empty((N, C), dtype=np.float32)
    for c in range(N_CORES):
        base = c * NP_CORE
        for half, r in ((0, res.results[c]["outA"]), (1, res.results[c]["outB"])):
            lo = base + half * NH
            hi = lo + NH
            s_recon[lo:hi] = r[0:54].T
            best_t[lo:hi] = r[54]
            texture[lo:hi] = r[55]
            best_e[lo:hi] = r[64:118].T
            count[lo:hi] = r[118]

    # Exact-tie repair: pixels where the one-hot had != 1 entries get the
    # reference argmin recomputed on host (count is exact: ties are rare).
    bad = np.flatnonzero(count != 1.0)
    if bad.size:
        d = s_obs[bad]
        s2 = np.sum(d.astype(np.float32) ** 2, axis=1, keepdims=True)
        losses = (s2 - 2.0 * (d @ cand.T) + c2[None, :]).astype(np.float32)
        kfix = np.argmin(losses, axis=1)
        s_recon[bad] = cand[kfix]
        best_t[bad] = t_cand[kfix % N_TEMPS]
        best_e[bad] = library[kfix // N_TEMPS]
        texture[bad] = library[kfix // N_TEMPS].mean(axis=1)

    best_v = np.full(N, 0.5, dtype=np.float32)
    beta = np.zeros((N, 1), dtype=np.float32)
    diff = s_obs.astype(np.float64) - s_recon.astype(np.float64)
    objective = np.float32(np.mean(np.sum(diff * diff, axis=1)))

    return best_t, best_e, best_v, beta, texture, s_recon, objective
